# revision 1
# baseline (speedup 1.0000x reference)
"""IoU loss kernel for Trainium2, data-parallel over 8 NeuronCores.

Math (per box, columns = x-center, y-center, half-size s):
    w = relu(min(x+s, x'+s') - max(x-s, x'-s'))
      = relu((s+s') - max(|x-x'|, |s-s'|))          # S - max identity
    h likewise with y.
    overlap = w*h
    union   = 4s^2 + 4s'^2 - overlap = 2(S^2 + D^2) - overlap,
              S = s+s', D = s-s'
    iou     = overlap / (union + 1e-7)
    loss    = -sum(log(iou + 1e-7));  iou_sum = sum(iou)

Engine split per 128x1024-box tile:
  DVE   : dx, dy, S, D (fp32 strided reads -> fp16), abs_max fusions,
          subs, relus (tensor_scalar, 4x), overlap, union,
          tensor_tensor_reduce for iou (+ per-tile iou partial sum)
  ACT   : squares via Square(sqrt2 * x), 1/(u+eps) via Exp(-Ln(u+eps)),
          final Ln(iou+eps) with accum_out giving the loss partial sum.
          All functions live in the natural_log_exp_and_others table set.
  Host  : final [128, 2T] x 8 cores partial-sum reduction in float64.
"""

import numpy as np

import concourse.bass as bass
import concourse.mybir as mybir
from concourse import tile
from concourse.bass_utils import run_bass_kernel_spmd

N = 8388608
NCORES = 8
NS = N // NCORES  # 1048576 boxes per core
P = 128
W = 1024          # boxes per partition per tile
T = NS // (P * W)  # 8 tiles per core
EPS = 1e-7
RT2 = 1.4142135623730951

F32 = mybir.dt.float32
F16 = mybir.dt.float16
Op = mybir.AluOpType
Act = mybir.ActivationFunctionType


def _build(T_: int = T, W_: int = W, compile_passes: bool = True) -> bass.Bass:
    from concourse import bacc

    ns = P * W_ * T_
    nc = bacc.Bacc()
    outs_d = nc.dram_tensor("outputs", [ns, 3], F32, kind="ExternalInput")
    tars_d = nc.dram_tensor("targets", [ns, 3], F32, kind="ExternalInput")
    acc_d = nc.dram_tensor("acc", [P, 2 * T_], F32, kind="ExternalOutput")

    outs_v = outs_d[:, :].rearrange("(t p w) c -> t p (w c)", t=T_, p=P, w=W_)
    tars_v = tars_d[:, :].rearrange("(t p w) c -> t p (w c)", t=T_, p=P, w=W_)
    T, W = T_, W_

    from concourse.tile_rust import add_dep_helper

    with tile.TileContext(nc) as tc:
        with tc.tile_pool(name="main", bufs=2) as pool:
            accs = pool.tile([P, 2 * T], F32, tag="accs", bufs=1)
            eps_t = pool.tile([P, 1], F32, tag="eps", bufs=1)
            nc.vector.memset(eps_t[:, :], EPS)
            last_ttr = None
            RAWBUFS = 4
            ttr_hist: list = []
            dmaO_hist: list = []
            dmaT_hist: list = []
            for t in range(T):
                rawO = pool.tile([P, 3 * W], F32, tag="rawO", bufs=RAWBUFS)
                rawT = pool.tile([P, 3 * W], F32, tag="rawT", bufs=RAWBUFS)
                if t >= RAWBUFS:
                    # DMA instructions have a single sync-wait slot, but a
                    # slot-recycling load needs a WAR wait (slot readers, DVE
                    # sem), a WAW wait, and a lane-reuse wait. With bufs=4 and
                    # 2 DMAs/tile the recycled slot's DMA sits exactly 8 DMAs
                    # back, so WAW and lane-reuse share one semaphore; a
                    # Drain (large wait budget) on the SP sequencer absorbs
                    # all conditions onto the SP-seq clock, leaving the big
                    # loads wait-free.
                    dr = nc.sync.drain(fusable=False)
                    add_dep_helper(dr.ins, ttr_hist[t - RAWBUFS].ins, sync=True,
                                   reason="absorb DVE WAR tick")
                    add_dep_helper(dr.ins, dmaO_hist[t - RAWBUFS].ins, sync=True,
                                   reason="absorb old rawO DMA lane")
                    add_dep_helper(dr.ins, dmaT_hist[t - RAWBUFS].ins, sync=True,
                                   reason="absorb old rawT DMA lane")
                dmaO_hist.append(nc.sync.dma_start(out=rawO[:, :], in_=outs_v[t]))
                dmaT_hist.append(nc.sync.dma_start(out=rawT[:, :], in_=tars_v[t]))
                # The TT ISA struct has a single sync-wait slot, but dx below
                # depends on BOTH input DMAs. Absorb rawT's semaphore with a
                # tiny copy so dx only needs the rawO wait.
                dummy = pool.tile([P, 1], F32, tag="dummy")
                nc.vector.tensor_copy(dummy[:, :], rawT[:, 0:1])
                o3 = rawO.rearrange("p (w c) -> p w c", c=3)
                t3 = rawT.rearrange("p (w c) -> p w c", c=3)
                x1, y1, s1 = o3[:, :, 0], o3[:, :, 1], o3[:, :, 2]
                x2, y2, s2 = t3[:, :, 0], t3[:, :, 1], t3[:, :, 2]

                dx = pool.tile([P, W], F16, tag="dx")
                nc.vector.tensor_tensor(dx[:, :], x1, x2, Op.subtract)
                dy = pool.tile([P, W], F16, tag="dy")
                nc.vector.tensor_tensor(dy[:, :], y1, y2, Op.subtract)
                S = pool.tile([P, W], F16, tag="S")
                nc.vector.tensor_tensor(S[:, :], s1, s2, Op.add)
                D = pool.tile([P, W], F16, tag="D")
                nc.vector.tensor_tensor(D[:, :], s1, s2, Op.subtract)

                # |dx|, |dy|, |D| on the scalar engine (abs_max is CoreSim-only)
                adx = pool.tile([P, W], F16, tag="adx")
                nc.scalar.activation(adx[:, :], dx[:, :], Act.Abs)
                ady = pool.tile([P, W], F16, tag="ady")
                nc.scalar.activation(ady[:, :], dy[:, :], Act.Abs)
                aD = pool.tile([P, W], F16, tag="aD")
                nc.scalar.activation(aD[:, :], D[:, :], Act.Abs)

                mw = pool.tile([P, W], F16, tag="mw")
                nc.vector.tensor_tensor(mw[:, :], adx[:, :], aD[:, :], Op.max)
                mh = pool.tile([P, W], F16, tag="mh")
                nc.vector.tensor_tensor(mh[:, :], ady[:, :], aD[:, :], Op.max)

                wr = pool.tile([P, W], F16, tag="wr")
                nc.vector.tensor_sub(wr[:, :], S[:, :], mw[:, :])
                hr = pool.tile([P, W], F16, tag="hr")
                nc.vector.tensor_sub(hr[:, :], S[:, :], mh[:, :])

                w_ = pool.tile([P, W], F16, tag="w_")
                nc.vector.tensor_scalar_max(w_[:, :], wr[:, :], 0.0)
                h_ = pool.tile([P, W], F16, tag="h_")
                nc.vector.tensor_scalar_max(h_[:, :], hr[:, :], 0.0)

                ov = pool.tile([P, W], F16, tag="ov")
                nc.vector.tensor_mul(ov[:, :], w_[:, :], h_[:, :])

                # 2*S^2 and 2*D^2 on the scalar engine: Square(sqrt2 * x)
                qS = pool.tile([P, W], F16, tag="qS")
                nc.scalar.activation(qS[:, :], S[:, :], Act.Square, scale=RT2)
                qD = pool.tile([P, W], F16, tag="qD")
                nc.scalar.activation(qD[:, :], D[:, :], Act.Square, scale=RT2)
                qs = pool.tile([P, W], F16, tag="qs")
                nc.vector.tensor_add(qs[:, :], qS[:, :], qD[:, :])

                ue = pool.tile([P, W], F16, tag="ue")
                nc.vector.tensor_sub(ue[:, :], qs[:, :], ov[:, :])

                # r = 1/(ue + eps) = exp(-ln(ue + eps)); fp32 (can reach 1e7)
                lnu = pool.tile([P, W], F32, tag="lnu")
                nc.scalar.activation(lnu[:, :], ue[:, :], Act.Ln, bias=eps_t[:, 0:1])
                r = pool.tile([P, W], F32, tag="r")
                nc.scalar.activation(r[:, :], lnu[:, :], Act.Exp, scale=-1.0)

                # iou = overlap * r, with running per-partition sum into accs[:, t]
                iou = pool.tile([P, W], F16, tag="iou")
                nc.vector.tensor_mul(iou[:, :], ov[:, :], r[:, :])
                last_ttr = nc.vector.tensor_reduce(
                    accs[:, t : t + 1], iou[:, :], mybir.AxisListType.X, Op.add
                )

                # loss partial: sum of Ln(iou + eps) via activation accumulate
                li = pool.tile([P, W], F32, tag="li")
                last_act = nc.scalar.activation(
                    li[:, :],
                    iou[:, :],
                    Act.Ln,
                    bias=eps_t[:, 0:1],
                    accum_out=accs[:, T + t : T + t + 1],
                )
                ttr_hist.append(last_ttr)

            # acc store would need waits on both the DVE (iou accums) and ACT
            # (loss accums) sems; absorb both on an SP drain first.
            dr = nc.sync.drain(fusable=False)
            add_dep_helper(dr.ins, last_ttr.ins, sync=True,
                           reason="absorb DVE accum tick before acc store")
            add_dep_helper(dr.ins, last_act.ins, sync=True,
                           reason="absorb ACT accum tick before acc store")
            nc.sync.dma_start(out=acc_d[:, :], in_=accs[:, :])

    if compile_passes:
        # Bacc.compile runs generate_event_semaphores (splits multi-wait
        # instructions to satisfy the 1-wait-per-instruction HW limit),
        # extended-inst lowering, and ACT table loads.
        nc.compile()
    return nc


_NC_CACHE: list[bass.Bass] = []


def _get_nc() -> bass.Bass:
    if not _NC_CACHE:
        _NC_CACHE.append(_build())
    return _NC_CACHE[0]


def _run(inputs: dict, trace: bool = False, trace_kwargs: dict | None = None):
    outputs = np.ascontiguousarray(np.asarray(inputs["outputs"], dtype=np.float32))
    targets = np.ascontiguousarray(np.asarray(inputs["targets"], dtype=np.float32))
    assert outputs.shape == (N, 3) and targets.shape == (N, 3)

    nc = _get_nc()
    in_maps = [
        {
            "outputs": outputs[c * NS : (c + 1) * NS],
            "targets": targets[c * NS : (c + 1) * NS],
        }
        for c in range(NCORES)
    ]
    kw = {}
    if trace:
        kw["trace"] = True
        if trace_kwargs:
            kw["trace_kwargs"] = trace_kwargs
    res = run_bass_kernel_spmd(nc, in_maps, list(range(NCORES)), **kw)

    iou_sum = 0.0
    loss = 0.0
    for c in range(NCORES):
        acc = np.asarray(res.results[c]["acc"], dtype=np.float64)
        iou_sum += acc[:, :T].sum()
        loss += acc[:, T:].sum()
    loss = -loss
    return (np.float32(loss), np.float32(iou_sum)), res


def kernel(**inputs) -> tuple:
    (loss, iou_sum), _ = _run(inputs)
    return (loss, iou_sum)



# revision 2
# speedup vs baseline: 1.0035x; 1.0035x over previous
"""IoU loss kernel for Trainium2, data-parallel over 8 NeuronCores (v2).

Math per box (columns x, y, half-size s):
    adx = |x1-x2|, ady = |y1-y2|, aD = |s1-s2|, S = s1+s2
    w = relu(S - max(adx, aD)),  h = relu(S - max(ady, aD))
    ov = w*h
    union = 4*s1^2 + 4*s2^2 - ov
    iou = ov / (union + eps)
    loss = -sum(ln(iou + eps));  iou_sum = sum(iou)

The union/overlap terms are carried x1024 (folded into ACT input
scales) so the reciprocal fits in f16 and the final iou multiply runs
in the DVE's 2x half-precision mode.

Engine split per [128 x 1024]-box tile (cost-model ns in brackets):
  SP  : two 1.5 MB contiguous DMA loads [2 x 4738] - the HBM roofline.
        Slot-recycling waits are absorbed on one drain block per 4
        tiles so the loads stay back-to-back.
  ACT : q1 = Square(64*s1), q2 = Square(64*s2) from strided raw f32,
        w' = Relu(1024*wr), h = Relu(hr), iou sum via Copy+accum_out,
        li = Ln(iou+eps)+accum_out -> loss partial [6 x 1061].
        Only square/relu/copy/ln funcs -> two act table loads total.
  DVE : 3 fused |a-b| customs + S (strided f32 reads) [4 x 1127],
        mw, mh, wr, hr, q (f16 2x stock) [5 x 593], ov' = w'*h [593],
        r'' ~= 1/(q - ov' + 1024eps) fused custom (bit-trick seed +
        1 Newton, ~0.4% worst-case) -> f16 [1127], iou = ov'*r''
        (f16 2x) [593].
  The (w', h, ov', r'', iou) tail of tile t is emitted one iteration
  late: cross-engine waits bind to the producer engine's instruction
  counter at emission time, so the lag keeps every wait satisfied
  before the in-order queues reach it.
  Host: final [128, 2*n_tiles] x 8 cores partial-sum reduction in f64.
"""

import numpy as np

import concourse.bass as bass
import concourse.mybir as mybir
from concourse import tile
from concourse.bass_utils import run_bass_kernel_spmd

N = 8388608
NCORES = 8
NS = N // NCORES  # 1048576 boxes per core
P = 128
W = 1024          # boxes per partition per full tile
T = NS // (P * W)  # 8 full tiles per core
EPS = 1e-7
SC = 1024.0       # carry overlap/union terms x1024 so 1/union fits f16

# Chebyshev seed constants from dve_ops.RECIP_APPROX_FAST_CONSTS.
RECIP_C0 = -0.23549792
RECIP_C1 = 2.0017324

F32 = mybir.dt.float32
F16 = mybir.dt.float16
Op = mybir.AluOpType
Act = mybir.ActivationFunctionType

_OPS_CACHE: dict = {}


def _ensure_custom_ops():
    """Register the two fused DVE ops (idempotent, keyed by name)."""
    if _OPS_CACHE:
        return _OPS_CACHE
    from concourse import dve_ops as D
    from concourse.dve_spec import (
        Spec, Src0, Src1, C0, C1, C2, AluOp, Bin, maxx, lower, _has_src1,
    )
    from concourse.dve_uop import DveOpSpec

    def reg(name, spec):
        if name in D._SUB_OPCODE_FOR_NAME:
            return next(o for o in D.OPS if o.name == name)
        row = D._CUSTOM_DVE_ROW_BASE + len(D.OPS)
        shas = {}
        for ver in ("v3", "v4"):
            try:
                s = DveOpSpec(
                    name=name, opcode=row, uops=lower(spec, ver=ver),
                    rd1_en=_has_src1(spec),
                )
                shas[ver] = s.sha(ver)
            except Exception:
                pass
        op = D.DveOp(name, spec, subdim=False, uops_sha=shas)
        D.OPS.append(op)
        D._SUB_OPCODE_FOR_NAME[name] = row
        D.CUSTOM_DVE_SPECS[name] = spec
        return op

    def ref_absdiff(in0, in1, c0, c1, c2):
        return np.abs(in0.astype(np.float32) - in1.astype(np.float32))

    def ref_recipdiff(in0, in1, c0, c1, c2):
        t = np.ascontiguousarray(
            in0.astype(np.float32) - in1.astype(np.float32) + np.float32(c0),
            dtype=np.float32,
        )
        nt = (~t.view(np.int32)).view(np.float32)
        y0 = nt * np.float32(c1)
        return y0 * (np.float32(c2) - t * y0)

    absdiff = reg(
        "IOU_ABSDIFF_V1",
        Spec(body=maxx(Src0 - Src1, Src1 - Src0), reference=ref_absdiff),
    )
    _t = (Src0 - Src1) + C0
    _nt = Bin(AluOp.BITWISE_NOT, _t, _t)
    _y0 = _nt * C1
    recipd = reg(
        "IOU_RECIPDIFF_V1",
        Spec(body=_y0 * (C2 - _t * _y0), reference=ref_recipdiff),
    )
    _OPS_CACHE.update(absdiff=absdiff, recipd=recipd)
    return _OPS_CACHE


def _build(T_: int = T, W_: int = W, reps: int = 1, loop_reps: int = 1,
           act_mini: bool = False, compile_passes: bool = True) -> bass.Bass:
    # act_mini: timing probe — shrink every ACT op to width 1 (wrong
    # numerics, identical instruction/sync structure) to expose whether
    # the Activation engine is the real-hardware bottleneck.
    from concourse import bacc
    from concourse.tile_rust import add_dep_helper

    ops = _ensure_custom_ops()
    absdiff, recipd = ops["absdiff"], ops["recipd"]

    ns = P * W_ * T_
    nc = bacc.Bacc()
    outs_d = nc.dram_tensor("outputs", [ns, 3], F32, kind="ExternalInput")
    tars_d = nc.dram_tensor("targets", [ns, 3], F32, kind="ExternalInput")

    # Variable tile schedule: small tiles at both ends shrink pipeline
    # fill (first compute starts after tile 0's two small DMAs) and the
    # post-last-DMA compute tail. Sizes sum to W_ * T_ columns.
    total_w = W_ * T_
    W_loc = W_
    if T_ >= 4:
        sched = ([W_ // 4, 3 * W_ // 4] + [W_] * (T_ - 2)
                 + [W_ // 2, W_ // 4, W_ // 4])
    else:
        sched = [W_] * T_
    assert sum(sched) == total_w, (sched, total_w)
    n_iter = len(sched)
    offs = [sum(sched[:i]) for i in range(n_iter)]

    acc_d = nc.dram_tensor("acc", [P, 2 * n_iter], F32, kind="ExternalOutput")
    T_loc = n_iter

    RAWBUFS = 5
    BATCH = 4  # drain/sem block cadence: DMAs stay adjacent within a batch

    with tile.TileContext(nc) as tc:
        with tc.tile_pool(name="main", bufs=2) as pool:
            accs = pool.tile([P, 2 * T_loc], F32, tag="accs", bufs=1)
            eps_t = pool.tile([P, 1], F32, tag="eps", bufs=1)
            nc.vector.memset(eps_t[:, :], EPS)
            dmaO_hist: list = []
            dmaT_hist: list = []
            S_hist: list = []
            q2_hist: list = []
            state = {"last_acc": None, "last_act": None}

            def relu_half(d):
                # ACT part of tile d's tail (emitted one iteration late,
                # before this iteration's DVE front so the DVE-counter
                # binding lands on tile d's hr, which is already done).
                wi = d["w"]
                w_ = pool.tile([P, W_loc], F16, tag="w_")
                nc.scalar.activation(w_[:, :wi], d["wr"][:, :wi], Act.Relu,
                                     scale=SC)
                h_ = pool.tile([P, W_loc], F16, tag="h_")
                nc.scalar.activation(h_[:, :wi], d["hr"][:, :wi], Act.Relu)
                d["w_"] = w_
                d["h_"] = h_

            def back_half(d):
                # DVE tail of tile d + ACT accumulation ops.
                wi = d["w"]
                t = d["t"]
                ov = pool.tile([P, W_loc], F16, tag="ov")
                nc.vector.tensor_tensor(ov[:, :wi], d["w_"][:, :wi],
                                        d["h_"][:, :wi], Op.mult)
                r = pool.tile([P, W_loc], F16, tag="r")
                nc.vector._custom_dve(recipd, out=r[:, :wi], in0=d["q"][:, :wi],
                                      in1=ov[:, :wi], s0=SC * EPS, s1=RECIP_C0,
                                      imm2=RECIP_C1)
                iou = pool.tile([P, W_loc], F16, tag="iou")
                last_iou = nc.vector.tensor_tensor(
                    iou[:, :wi], ov[:, :wi], r[:, :wi], Op.mult)
                # ACT: iou sum via Copy+accum, loss partial via Ln+accum.
                cp = pool.tile([P, W_loc], F16, tag="cp")
                state["last_acc"] = nc.scalar.activation(
                    cp[:, :wi], iou[:, :wi], Act.Copy,
                    accum_out=accs[:, t : t + 1],
                )
                li = pool.tile([P, W_loc], F32, tag="li")
                state["last_act"] = nc.scalar.activation(
                    li[:, :wi], iou[:, :wi], Act.Ln, bias=eps_t[:, 0:1],
                    accum_out=accs[:, T_loc + t : T_loc + t + 1],
                )
                return last_iou

            def emit_span(span_iters):
                prev = None
                for idx in range(span_iters):
                    prev = emit_iter(idx, span_iters, prev)
                relu_half(prev)
                back_half(prev)

            def emit_iter(idx, span_iters, prev):
                t = idx % n_iter
                wi = sched[t]
                off = offs[t]
                ov_view = outs_d[P * off : P * (off + wi), :].rearrange(
                    "(p w) c -> p (w c)", p=P, w=wi)
                tv_view = tars_d[P * off : P * (off + wi), :].rearrange(
                    "(p w) c -> p (w c)", p=P, w=wi)
                rawO = pool.tile([P, 3 * W_loc], F32, tag="rawO", bufs=RAWBUFS)
                rawT = pool.tile([P, 3 * W_loc], F32, tag="rawT", bufs=RAWBUFS)
                if idx >= RAWBUFS and idx % BATCH == 0:
                    # Recycled-slot DMAs need WAR waits on the slots' last
                    # readers (DVE: S covers the absdiffs in-order; ACT: q2
                    # covers q1), WAW waits, and lane-reuse waits. The cost
                    # model only overlaps a DMA's ~1.7us descriptor-gen init
                    # with the previous transfer when DMAs are adjacent in
                    # the stream, so absorb the waits for a whole BATCH of
                    # upcoming loads on one drain/sem block.
                    dr = nc.sync.drain(fusable=False)
                    for j in range(idx - RAWBUFS,
                                   min(idx - RAWBUFS + BATCH,
                                       span_iters - RAWBUFS)):
                        add_dep_helper(dr.ins, S_hist[j].ins, sync=True,
                                       reason="absorb DVE WAR tick")
                        add_dep_helper(dr.ins, q2_hist[j].ins, sync=True,
                                       reason="absorb ACT WAR tick")
                        add_dep_helper(dr.ins, dmaO_hist[j].ins, sync=True,
                                       reason="absorb old rawO DMA lane")
                        add_dep_helper(dr.ins, dmaT_hist[j].ins, sync=True,
                                       reason="absorb old rawT DMA lane")
                dmaO_hist.append(
                    nc.sync.dma_start(out=rawO[:, : 3 * wi], in_=ov_view))
                dmaT_hist.append(
                    nc.sync.dma_start(out=rawT[:, : 3 * wi], in_=tv_view))

                o3 = rawO[:, : 3 * wi].rearrange("p (w c) -> p w c", c=3)
                t3 = rawT[:, : 3 * wi].rearrange("p (w c) -> p w c", c=3)
                x1, y1, s1 = o3[:, :, 0], o3[:, :, 1], o3[:, :, 2]
                x2, y2, s2 = t3[:, :, 0], t3[:, :, 1], t3[:, :, 2]

                # ACT: squares of the strided size columns: 1024*4s^2.
                q1 = pool.tile([P, W_loc], F16, tag="q1")
                nc.scalar.activation(q1[:, :wi], s1, Act.Square, scale=64.0)
                q2 = pool.tile([P, W_loc], F16, tag="q2")
                q2_hist.append(
                    nc.scalar.activation(q2[:, :wi], s2, Act.Square, scale=64.0)
                )
                # ACT relus of the previous tile's wr/hr.
                if prev is not None:
                    relu_half(prev)

                # DVE: absorb rawT's DMA semaphore with a tiny copy so the
                # 2-input customs below only need the rawO wait.
                dummy = pool.tile([P, 1], F32, tag="dummy")
                nc.vector.tensor_copy(dummy[:, :], rawT[:, 0:1])

                adx = pool.tile([P, W_loc], F16, tag="adx")
                nc.vector._custom_dve(absdiff, out=adx[:, :wi], in0=x1, in1=x2)
                aD = pool.tile([P, W_loc], F16, tag="aD")
                nc.vector._custom_dve(absdiff, out=aD[:, :wi], in0=s1, in1=s2)
                ady = pool.tile([P, W_loc], F16, tag="ady")
                nc.vector._custom_dve(absdiff, out=ady[:, :wi], in0=y1, in1=y2)
                S = pool.tile([P, W_loc], F16, tag="S")
                S_hist.append(
                    nc.vector.tensor_tensor(S[:, :wi], s1, s2, Op.add))

                mw = pool.tile([P, W_loc], F16, tag="mw")
                nc.vector.tensor_tensor(mw[:, :wi], adx[:, :wi], aD[:, :wi],
                                        Op.max)
                mh = pool.tile([P, W_loc], F16, tag="mh")
                nc.vector.tensor_tensor(mh[:, :wi], ady[:, :wi], aD[:, :wi],
                                        Op.max)
                wr = pool.tile([P, W_loc], F16, tag="wr", bufs=3)
                nc.vector.tensor_tensor(wr[:, :wi], S[:, :wi], mw[:, :wi],
                                        Op.subtract)
                hr = pool.tile([P, W_loc], F16, tag="hr", bufs=3)
                nc.vector.tensor_tensor(hr[:, :wi], S[:, :wi], mh[:, :wi],
                                        Op.subtract)
                q = pool.tile([P, W_loc], F16, tag="q", bufs=3)
                nc.vector.tensor_tensor(q[:, :wi], q1[:, :wi], q2[:, :wi],
                                        Op.add)

                # DVE+ACT tail of the previous tile.
                if prev is not None:
                    back_half(prev)

                return {"t": t, "w": wi, "wr": wr, "hr": hr, "q": q}

            if loop_reps > 1:
                # Hardware loop for the differential timing harness: one
                # full pass per iteration, ~2us back-edge barrier between
                # passes, so hundreds of passes fit one small NEFF.
                with tc.For_i(0, loop_reps, 1):
                    dmaO_hist.clear()
                    dmaT_hist.clear()
                    S_hist.clear()
                    q2_hist.clear()
                    emit_span(n_iter)
            else:
                emit_span(reps * n_iter)
            last_acc = state["last_acc"]
            last_act = state["last_act"]

            # acc store needs waits on the ACT accum sems; absorb on an SP
            # drain first (both accums are ACT ops, the later covers).
            dr = nc.sync.drain(fusable=False)
            add_dep_helper(dr.ins, last_acc.ins, sync=True,
                           reason="absorb ACT iou-accum tick before store")
            add_dep_helper(dr.ins, last_act.ins, sync=True,
                           reason="absorb ACT ln-accum tick before store")
            nc.sync.dma_start(out=acc_d[:, :], in_=accs[:, :])

    if compile_passes:
        nc.compile()
    return nc


_NC_CACHE: list = []


def _get_nc() -> bass.Bass:
    if not _NC_CACHE:
        _NC_CACHE.append(_build())
    return _NC_CACHE[0]


def _run(inputs: dict, trace: bool = False, trace_kwargs: dict | None = None):
    outputs = np.ascontiguousarray(np.asarray(inputs["outputs"], dtype=np.float32))
    targets = np.ascontiguousarray(np.asarray(inputs["targets"], dtype=np.float32))
    assert outputs.shape == (N, 3) and targets.shape == (N, 3)

    nc = _get_nc()
    in_maps = [
        {
            "outputs": outputs[c * NS : (c + 1) * NS],
            "targets": targets[c * NS : (c + 1) * NS],
        }
        for c in range(NCORES)
    ]
    kw = {}
    if trace:
        kw["trace"] = True
        if trace_kwargs:
            kw["trace_kwargs"] = trace_kwargs
    res = run_bass_kernel_spmd(nc, in_maps, list(range(NCORES)), **kw)

    iou_sum = 0.0
    loss = 0.0
    for c in range(NCORES):
        acc = np.asarray(res.results[c]["acc"], dtype=np.float64)
        half = acc.shape[1] // 2
        iou_sum += acc[:, :half].sum()
        loss += acc[:, half:].sum()
    loss = -loss
    return (np.float32(loss), np.float32(iou_sum)), res


def kernel(**inputs) -> tuple:
    (loss, iou_sum), _ = _run(inputs)
    return (loss, iou_sum)


# revision 3
# speedup vs baseline: 1.0105x; 1.0070x over previous
"""IoU loss kernel for Trainium2, data-parallel over 8 NeuronCores (v2).

Math per box (columns x, y, half-size s):
    adx = |x1-x2|, ady = |y1-y2|, aD = |s1-s2|, S = s1+s2
    w = relu(S - max(adx, aD)),  h = relu(S - max(ady, aD))
    ov = w*h
    union = 4*s1^2 + 4*s2^2 - ov
    iou = ov / (union + eps)
    loss = -sum(ln(iou + eps));  iou_sum = sum(iou)

The union/overlap terms are carried x1024 (folded into ACT input
scales) so the reciprocal fits in f16 and the final iou multiply runs
in the DVE's 2x half-precision mode.

Engine split per [128 x 1024]-box tile (cost-model ns in brackets):
  SP  : two 1.5 MB contiguous DMA loads [2 x 4738] - the HBM roofline.
        Slot-recycling waits are absorbed on one drain block per 4
        tiles so the loads stay back-to-back.
  ACT : q1 = Square(64*s1), q2 = Square(64*s2) from strided raw f32,
        w' = Relu(1024*wr), h = Relu(hr), iou sum via Copy+accum_out,
        li = Ln(iou+eps)+accum_out -> loss partial [6 x 1061].
        Only square/relu/copy/ln funcs -> two act table loads total.
  DVE : 3 fused |a-b| customs + S (strided f32 reads) [4 x 1127],
        mw, mh, wr, hr, q (f16 2x stock) [5 x 593], ov' = w'*h [593],
        r'' ~= 1/(q - ov' + 1024eps) fused custom (bit-trick seed +
        1 Newton, ~0.4% worst-case) -> f16 [1127], iou = ov'*r''
        (f16 2x) [593].
  The (w', h, ov', r'', iou) tail of tile t is emitted one iteration
  late: cross-engine waits bind to the producer engine's instruction
  counter at emission time, so the lag keeps every wait satisfied
  before the in-order queues reach it.
  Host: final [128, 2*n_tiles] x 8 cores partial-sum reduction in f64.
"""

import numpy as np

import concourse.bass as bass
import concourse.mybir as mybir
from concourse import tile
from concourse.bass_utils import run_bass_kernel_spmd

N = 8388608
NCORES = 8
NS = N // NCORES  # 1048576 boxes per core
P = 128
W = 1024          # boxes per partition per full tile
T = NS // (P * W)  # 8 full tiles per core
EPS = 1e-7
SC = 1024.0       # carry overlap/union terms x1024 so 1/union fits f16

# Chebyshev seed constants from dve_ops.RECIP_APPROX_FAST_CONSTS.
RECIP_C0 = -0.23549792
RECIP_C1 = 2.0017324

F32 = mybir.dt.float32
F16 = mybir.dt.float16
Op = mybir.AluOpType
Act = mybir.ActivationFunctionType

_OPS_CACHE: dict = {}


def _ensure_custom_ops():
    """Register the two fused DVE ops (idempotent, keyed by name)."""
    if _OPS_CACHE:
        return _OPS_CACHE
    from concourse import dve_ops as D
    from concourse.dve_spec import (
        Spec, Src0, Src1, C0, C1, C2, AluOp, Bin, maxx, lower, _has_src1,
    )
    from concourse.dve_uop import DveOpSpec

    def reg(name, spec):
        if name in D._SUB_OPCODE_FOR_NAME:
            return next(o for o in D.OPS if o.name == name)
        row = D._CUSTOM_DVE_ROW_BASE + len(D.OPS)
        shas = {}
        for ver in ("v3", "v4"):
            try:
                s = DveOpSpec(
                    name=name, opcode=row, uops=lower(spec, ver=ver),
                    rd1_en=_has_src1(spec),
                )
                shas[ver] = s.sha(ver)
            except Exception:
                pass
        op = D.DveOp(name, spec, subdim=False, uops_sha=shas)
        D.OPS.append(op)
        D._SUB_OPCODE_FOR_NAME[name] = row
        D.CUSTOM_DVE_SPECS[name] = spec
        return op

    def ref_absdiff(in0, in1, c0, c1, c2):
        return np.abs(in0.astype(np.float32) - in1.astype(np.float32))

    def ref_recipdiff(in0, in1, c0, c1, c2):
        t = np.ascontiguousarray(
            in0.astype(np.float32) - in1.astype(np.float32) + np.float32(c0),
            dtype=np.float32,
        )
        nt = (~t.view(np.int32)).view(np.float32)
        y0 = nt * np.float32(c1)
        return y0 * (np.float32(c2) - t * y0)

    absdiff = reg(
        "IOU_ABSDIFF_V1",
        Spec(body=maxx(Src0 - Src1, Src1 - Src0), reference=ref_absdiff),
    )
    _t = (Src0 - Src1) + C0
    _nt = Bin(AluOp.BITWISE_NOT, _t, _t)
    _y0 = _nt * C1
    recipd = reg(
        "IOU_RECIPDIFF_V1",
        Spec(body=_y0 * (C2 - _t * _y0), reference=ref_recipdiff),
    )
    _OPS_CACHE.update(absdiff=absdiff, recipd=recipd)
    return _OPS_CACHE


def _build(T_: int = T, W_: int = W, reps: int = 1, loop_reps: int = 1,
           act_mini: bool = False, compile_passes: bool = True) -> bass.Bass:
    # act_mini: timing probe — shrink every ACT op to width 1 (wrong
    # numerics, identical instruction/sync structure) to expose whether
    # the Activation engine is the real-hardware bottleneck.
    from concourse import bacc
    from concourse.tile_rust import add_dep_helper

    ops = _ensure_custom_ops()
    absdiff, recipd = ops["absdiff"], ops["recipd"]

    ns = P * W_ * T_
    nc = bacc.Bacc()
    outs_d = nc.dram_tensor("outputs", [ns, 3], F32, kind="ExternalInput")
    tars_d = nc.dram_tensor("targets", [ns, 3], F32, kind="ExternalInput")

    # Variable tile schedule: small tiles at both ends shrink pipeline
    # fill (first compute starts after tile 0's two small DMAs) and the
    # post-last-DMA compute tail. Sizes sum to W_ * T_ columns.
    total_w = W_ * T_
    W_loc = W_
    if T_ >= 4:
        # Uniform except a split last tile: fewer DMAs (each has ~1us
        # fixed cost on the real ring) while keeping the drain tail short.
        sched = [W_] * (T_ - 1) + [W_ // 2, W_ // 4, W_ // 4]
    else:
        sched = [W_] * T_
    assert sum(sched) == total_w, (sched, total_w)
    n_iter = len(sched)
    offs = [sum(sched[:i]) for i in range(n_iter)]

    acc_d = nc.dram_tensor("acc", [P, 2 * n_iter], F32, kind="ExternalOutput")
    T_loc = n_iter

    RAWBUFS = 5
    BATCH = 4  # drain/sem block cadence: DMAs stay adjacent within a batch

    with tile.TileContext(nc) as tc:
        with tc.tile_pool(name="main", bufs=2) as pool:
            accs = pool.tile([P, 2 * T_loc], F32, tag="accs", bufs=1)
            eps_t = pool.tile([P, 1], F32, tag="eps", bufs=1)
            nc.vector.memset(eps_t[:, :], EPS)
            dmaO_hist: list = []
            dmaT_hist: list = []
            S_hist: list = []
            q2_hist: list = []
            state = {"last_acc": None, "last_act": None}

            def relu_half(d):
                # ACT part of tile d's tail (emitted one iteration late,
                # before this iteration's DVE front so the DVE-counter
                # binding lands on tile d's hr, which is already done).
                wi = d["w"]
                w_ = pool.tile([P, W_loc], F16, tag="w_")
                nc.scalar.activation(w_[:, :wi], d["wr"][:, :wi], Act.Relu,
                                     scale=SC)
                h_ = pool.tile([P, W_loc], F16, tag="h_")
                nc.scalar.activation(h_[:, :wi], d["hr"][:, :wi], Act.Relu)
                d["w_"] = w_
                d["h_"] = h_

            def back_half(d):
                # DVE tail of tile d + ACT accumulation ops.
                wi = d["w"]
                t = d["t"]
                ov = pool.tile([P, W_loc], F16, tag="ov")
                nc.vector.tensor_tensor(ov[:, :wi], d["w_"][:, :wi],
                                        d["h_"][:, :wi], Op.mult)
                r = pool.tile([P, W_loc], F16, tag="r")
                nc.vector._custom_dve(recipd, out=r[:, :wi], in0=d["q"][:, :wi],
                                      in1=ov[:, :wi], s0=SC * EPS, s1=RECIP_C0,
                                      imm2=RECIP_C1)
                iou = pool.tile([P, W_loc], F16, tag="iou")
                last_iou = nc.vector.tensor_tensor(
                    iou[:, :wi], ov[:, :wi], r[:, :wi], Op.mult)
                # ACT: iou sum via Copy+accum, loss partial via Ln+accum.
                cp = pool.tile([P, W_loc], F16, tag="cp")
                state["last_acc"] = nc.scalar.activation(
                    cp[:, :wi], iou[:, :wi], Act.Copy,
                    accum_out=accs[:, t : t + 1],
                )
                li = pool.tile([P, W_loc], F32, tag="li")
                state["last_act"] = nc.scalar.activation(
                    li[:, :wi], iou[:, :wi], Act.Ln, bias=eps_t[:, 0:1],
                    accum_out=accs[:, T_loc + t : T_loc + t + 1],
                )
                return last_iou

            def emit_span(span_iters):
                prev = None
                for idx in range(span_iters):
                    prev = emit_iter(idx, span_iters, prev)
                relu_half(prev)
                back_half(prev)

            def emit_iter(idx, span_iters, prev):
                t = idx % n_iter
                wi = sched[t]
                off = offs[t]
                ov_view = outs_d[P * off : P * (off + wi), :].rearrange(
                    "(p w) c -> p (w c)", p=P, w=wi)
                tv_view = tars_d[P * off : P * (off + wi), :].rearrange(
                    "(p w) c -> p (w c)", p=P, w=wi)
                rawO = pool.tile([P, 3 * W_loc], F32, tag="rawO", bufs=RAWBUFS)
                rawT = pool.tile([P, 3 * W_loc], F32, tag="rawT", bufs=RAWBUFS)
                if idx >= RAWBUFS and idx % BATCH == 0:
                    # Recycled-slot DMAs need WAR waits on the slots' last
                    # readers (DVE: S covers the absdiffs in-order; ACT: q2
                    # covers q1), WAW waits, and lane-reuse waits. The cost
                    # model only overlaps a DMA's ~1.7us descriptor-gen init
                    # with the previous transfer when DMAs are adjacent in
                    # the stream, so absorb the waits for a whole BATCH of
                    # upcoming loads on one drain/sem block.
                    dr = nc.sync.drain(fusable=False)
                    for j in range(idx - RAWBUFS,
                                   min(idx - RAWBUFS + BATCH,
                                       span_iters - RAWBUFS)):
                        add_dep_helper(dr.ins, S_hist[j].ins, sync=True,
                                       reason="absorb DVE WAR tick")
                        add_dep_helper(dr.ins, q2_hist[j].ins, sync=True,
                                       reason="absorb ACT WAR tick")
                        add_dep_helper(dr.ins, dmaO_hist[j].ins, sync=True,
                                       reason="absorb old rawO DMA lane")
                        add_dep_helper(dr.ins, dmaT_hist[j].ins, sync=True,
                                       reason="absorb old rawT DMA lane")
                dmaO_hist.append(
                    nc.sync.dma_start(out=rawO[:, : 3 * wi], in_=ov_view))
                dmaT_hist.append(
                    nc.sync.dma_start(out=rawT[:, : 3 * wi], in_=tv_view))

                o3 = rawO[:, : 3 * wi].rearrange("p (w c) -> p w c", c=3)
                t3 = rawT[:, : 3 * wi].rearrange("p (w c) -> p w c", c=3)
                x1, y1, s1 = o3[:, :, 0], o3[:, :, 1], o3[:, :, 2]
                x2, y2, s2 = t3[:, :, 0], t3[:, :, 1], t3[:, :, 2]

                # ACT: squares of the strided size columns: 1024*4s^2.
                q1 = pool.tile([P, W_loc], F16, tag="q1")
                nc.scalar.activation(q1[:, :wi], s1, Act.Square, scale=64.0)
                q2 = pool.tile([P, W_loc], F16, tag="q2")
                q2_hist.append(
                    nc.scalar.activation(q2[:, :wi], s2, Act.Square, scale=64.0)
                )
                # ACT relus of the previous tile's wr/hr.
                if prev is not None:
                    relu_half(prev)

                # DVE: absorb rawT's DMA semaphore with a tiny copy so the
                # 2-input customs below only need the rawO wait.
                dummy = pool.tile([P, 1], F32, tag="dummy")
                nc.vector.tensor_copy(dummy[:, :], rawT[:, 0:1])

                adx = pool.tile([P, W_loc], F16, tag="adx")
                nc.vector._custom_dve(absdiff, out=adx[:, :wi], in0=x1, in1=x2)
                aD = pool.tile([P, W_loc], F16, tag="aD")
                nc.vector._custom_dve(absdiff, out=aD[:, :wi], in0=s1, in1=s2)
                ady = pool.tile([P, W_loc], F16, tag="ady")
                nc.vector._custom_dve(absdiff, out=ady[:, :wi], in0=y1, in1=y2)
                S = pool.tile([P, W_loc], F16, tag="S")
                S_hist.append(
                    nc.vector.tensor_tensor(S[:, :wi], s1, s2, Op.add))

                mw = pool.tile([P, W_loc], F16, tag="mw")
                nc.vector.tensor_tensor(mw[:, :wi], adx[:, :wi], aD[:, :wi],
                                        Op.max)
                mh = pool.tile([P, W_loc], F16, tag="mh")
                nc.vector.tensor_tensor(mh[:, :wi], ady[:, :wi], aD[:, :wi],
                                        Op.max)
                wr = pool.tile([P, W_loc], F16, tag="wr", bufs=3)
                nc.vector.tensor_tensor(wr[:, :wi], S[:, :wi], mw[:, :wi],
                                        Op.subtract)
                hr = pool.tile([P, W_loc], F16, tag="hr", bufs=3)
                nc.vector.tensor_tensor(hr[:, :wi], S[:, :wi], mh[:, :wi],
                                        Op.subtract)
                q = pool.tile([P, W_loc], F16, tag="q", bufs=3)
                nc.vector.tensor_tensor(q[:, :wi], q1[:, :wi], q2[:, :wi],
                                        Op.add)

                # DVE+ACT tail of the previous tile.
                if prev is not None:
                    back_half(prev)

                return {"t": t, "w": wi, "wr": wr, "hr": hr, "q": q}

            if loop_reps > 1:
                # Hardware loop for the differential timing harness: one
                # full pass per iteration, ~2us back-edge barrier between
                # passes, so hundreds of passes fit one small NEFF.
                with tc.For_i(0, loop_reps, 1):
                    dmaO_hist.clear()
                    dmaT_hist.clear()
                    S_hist.clear()
                    q2_hist.clear()
                    emit_span(n_iter)
            else:
                emit_span(reps * n_iter)
            last_acc = state["last_acc"]
            last_act = state["last_act"]

            # acc store needs waits on the ACT accum sems; absorb on an SP
            # drain first (both accums are ACT ops, the later covers).
            dr = nc.sync.drain(fusable=False)
            add_dep_helper(dr.ins, last_acc.ins, sync=True,
                           reason="absorb ACT iou-accum tick before store")
            add_dep_helper(dr.ins, last_act.ins, sync=True,
                           reason="absorb ACT ln-accum tick before store")
            nc.sync.dma_start(out=acc_d[:, :], in_=accs[:, :])

    if compile_passes:
        nc.compile()
    return nc


_NC_CACHE: list = []


def _get_nc() -> bass.Bass:
    if not _NC_CACHE:
        _NC_CACHE.append(_build())
    return _NC_CACHE[0]


def _run(inputs: dict, trace: bool = False, trace_kwargs: dict | None = None):
    outputs = np.ascontiguousarray(np.asarray(inputs["outputs"], dtype=np.float32))
    targets = np.ascontiguousarray(np.asarray(inputs["targets"], dtype=np.float32))
    assert outputs.shape == (N, 3) and targets.shape == (N, 3)

    nc = _get_nc()
    in_maps = [
        {
            "outputs": outputs[c * NS : (c + 1) * NS],
            "targets": targets[c * NS : (c + 1) * NS],
        }
        for c in range(NCORES)
    ]
    kw = {}
    if trace:
        kw["trace"] = True
        if trace_kwargs:
            kw["trace_kwargs"] = trace_kwargs
    res = run_bass_kernel_spmd(nc, in_maps, list(range(NCORES)), **kw)

    iou_sum = 0.0
    loss = 0.0
    for c in range(NCORES):
        acc = np.asarray(res.results[c]["acc"], dtype=np.float64)
        half = acc.shape[1] // 2
        iou_sum += acc[:, :half].sum()
        loss += acc[:, half:].sum()
    loss = -loss
    return (np.float32(loss), np.float32(iou_sum)), res


def kernel(**inputs) -> tuple:
    (loss, iou_sum), _ = _run(inputs)
    return (loss, iou_sum)
